# revision 11
# baseline (speedup 1.0000x reference)
"""Mirostat sampler Trainium2 kernel.

Contract: kernel(logits, u) -> int32 [64] token indices, matching
    reference: softmax -> desc sort -> cumsum -> top-p(0.9) mask -> renorm
    -> Gumbel-max sample -> map back through sort order.

Strategy (per core, 8 rows of the batch):
  - The argmax of log(trunc_p) + gumbel over sorted ranks equals the argmax of
    sorted_logit + gumbel over kept ranks (monotone shift), so the sort key is
    the raw logit and no transcendental enters the ordering.
  - Value-only bitonic sort of logits (desc) per row laid out as
    [16 partitions x 8192 cols] (8 rows -> 128 partitions), linear index
    l = p*8192 + c, pads -3e38 at the tail.
  - exp/cumsum of sorted values gives the top-p cutoff mask (cum <= 0.9*Z).
  - score = sorted_logit + gumbel(u[rank]) on kept ranks; global argmax;
    winner value x*; token recovered in original token space by counting
    #{x > x*} and selecting the (r* - count)-th smallest-index token among
    x == x* (exact stable-sort tie semantics).
"""

import sys

sys.path.insert(0, "/opt/trn_rl_repo")

import numpy as np  # noqa: E402

B, V = 64, 128000
NROW = 8            # batch rows per core
GP = 16             # partitions per row-group
CW = 8192           # columns
N = GP * CW         # 131072 padded row length
NEG = -3.0e38
NCORES = 8

_f32 = None  # set to mybir.dt.float32 lazily


def _emit(nc, tc, ctx):
    import concourse.mybir as mybir
    from concourse import bass

    f32 = mybir.dt.float32
    i32 = mybir.dt.int32
    Alu = mybir.AluOpType
    Act = mybir.ActivationFunctionType

    x_d = nc.dram_tensor("x", [NROW, V], f32, kind="ExternalInput").ap()
    u_d = nc.dram_tensor("u", [NROW, V], f32, kind="ExternalInput").ap()
    tok_d = nc.dram_tensor("tok", [128, 1], f32, kind="ExternalOutput").ap()
    dbg_d = nc.dram_tensor("dbg", [128, 8], f32, kind="ExternalOutput").ap()

    pool = ctx.enter_context(tc.tile_pool(name="main", bufs=1))

    # --- big tiles (5 x 32KB/partition) ---
    tA = pool.tile([128, CW], f32, tag="tA")   # sort ping
    tB = pool.tile([128, CW], f32, tag="tB")   # sort pong (loaded first)
    tC = pool.tile([128, CW], f32, tag="tC")   # shuffle scratch / e / km / idx
    tD = pool.tile([128, CW], f32, tag="tD")   # original x (token layout)
    tE = pool.tile([128, CW], f32, tag="tE")   # u -> gumbel -> score / scratch

    # --- small tiles ---
    sm = ctx.enter_context(tc.tile_pool(name="small", bufs=1))
    s_sign = {d: sm.tile([128, 1], f32, tag=f"sign{d}", name=f"sign{d}") for d in (1, 2, 4, 8)}
    s_flip = {kb: sm.tile([128, 1], f32, tag=f"flip{kb}", name=f"flip{kb}") for kb in (2, 4, 8, 16)}
    s_cmask = {s: sm.tile([128, 1], f32, tag=f"cm{s}", name=f"cm{s}") for s in (1, 2, 4, 8)}
    s_rowbase = sm.tile([128, 1], f32, tag="rowbase")
    s_eps = sm.tile([128, 1], f32, tag="eps")
    nc.vector.memset(s_eps[:], 1e-20)
    sc = {k: sm.tile([128, 1], f32, tag=f"sc_{k}", name=f"sc_{k}") for k in
          ("m", "negm", "tot", "pref", "carry", "Z", "thr", "M", "rstar",
           "xstar", "cnt", "jstar", "tokv", "t0", "t1", "t2")}

    # host-provided constants
    consts_d = nc.dram_tensor("consts", [128, 16], f32, kind="ExternalInput").ap()
    cst = sm.tile([128, 16], f32, tag="cst")
    nc.sync.dma_start(out=cst[:], in_=consts_d)
    # const layout (columns): 0..3 sign d=1,2,4,8 ; 4..7 flip kb=2,4,8,16 ;
    # 8..11 carrymask s=1,2,4,8 ; 12 rowbase(=g*CW*GP? see host) ;
    for j, d in enumerate((1, 2, 4, 8)):
        nc.vector.tensor_copy(s_sign[d][:], cst[:, j:j + 1])
    for j, kb in enumerate((2, 4, 8, 16)):
        nc.vector.tensor_copy(s_flip[kb][:], cst[:, 4 + j:5 + j])
    for j, s in enumerate((1, 2, 4, 8)):
        nc.vector.tensor_copy(s_cmask[s][:], cst[:, 8 + j:9 + j])
    nc.vector.tensor_copy(s_rowbase[:], cst[:, 12:13])

    # --- load inputs ---
    nc.vector.memset(tB[:], NEG)
    nc.vector.memset(tD[:], NEG)
    nc.vector.memset(tE[:], 0.5)
    MAIN = (GP - 1) * CW  # 122880
    TAIL = V - MAIN       # 5120
    for g in range(NROW):
        p0 = g * GP
        for dst, src in ((tB, x_d), (tD, x_d), (tE, u_d)):
            nc.sync.dma_start(
                out=dst[:][p0:p0 + GP - 1, :],
                in_=src[g, 0:MAIN].rearrange("(q c) -> q c", c=CW),
            )
            nc.sync.dma_start(
                out=dst[:][p0 + GP - 1:p0 + GP, 0:TAIL],
                in_=src[g:g + 1, MAIN:V],
            )

    # helpers -----------------------------------------------------------
    def shuffle_mask(fn):
        return [fn(i) for i in range(32)]

    def bfly(dst, src, op, tmp):
        """XOR-butterfly all-reduce over each 16-partition group, [128,1]."""
        if dst is not src:
            nc.vector.tensor_copy(dst[:], src[:])
        for s in (1, 2, 4, 8):
            nc.vector.stream_shuffle(tmp[:], dst[:], shuffle_mask(lambda i: i ^ s))
            nc.vector.tensor_tensor(out=dst[:], in0=dst[:], in1=tmp[:], op=op)

    def bcast(tile_col):  # [128,1] -> broadcast along free dim
        return tile_col[:].to_broadcast([128, CW])

    # --- bitonic sort (desc) on tB -> ping-pong with tA ---------------
    cur, nxt = tB, tA

    def free_straight(j):
        nonlocal cur, nxt
        a = cur[:].rearrange("p (b two j) -> p b two j", two=2, j=j)
        o = nxt[:].rearrange("p (b two j) -> p b two j", two=2, j=j)
        nc.vector.tensor_tensor(out=o[:, :, 0, :], in0=a[:, :, 0, :],
                                in1=a[:, :, 1, :], op=Alu.max)
        nc.vector.tensor_tensor(out=o[:, :, 1, :], in0=a[:, :, 0, :],
                                in1=a[:, :, 1, :], op=Alu.min)
        cur, nxt = nxt, cur

    def free_flip(k):
        nonlocal cur, nxt
        a = cur[:].rearrange("p (b k) -> p b k", k=k)
        o = nxt[:].rearrange("p (b k) -> p b k", k=k)
        h = k // 2
        lo_f = a[:, :, 0:h]
        hi_r = a[:, :, k - 1:h - 1:-1] if h >= 1 else None  # reversed 2nd half
        nc.vector.tensor_tensor(out=o[:, :, 0:h], in0=lo_f, in1=hi_r, op=Alu.max)
        hi_f = a[:, :, h:k]
        lo_r = a[:, :, h - 1::-1]                           # reversed 1st half
        nc.vector.tensor_tensor(out=o[:, :, h:k], in0=hi_f, in1=lo_r, op=Alu.min)
        cur, nxt = nxt, cur

    def part_sub(sign_col, perm, rev_free):
        """generic partition substage: out = s * max(s*cur, -perm(s*cur))."""
        nonlocal cur, nxt
        # t1 = cur * s  -> nxt
        nc.vector.tensor_tensor(out=nxt[:], in0=cur[:], in1=bcast(sign_col),
                                op=Alu.mult)
        # sh = shuffle(t1) -> tC
        nc.vector.stream_shuffle(tC[:], nxt[:], perm)
        sh_ap = tC[:, CW - 1::-1] if rev_free else tC[:]
        # q = max(t1, -sh) -> nxt (in1 aligned with out)
        nc.vector.scalar_tensor_tensor(out=nxt[:], in0=sh_ap, scalar=-1.0,
                                       in1=nxt[:], op0=Alu.mult, op1=Alu.max)
        # out = q * s
        nc.vector.tensor_tensor(out=nxt[:], in0=nxt[:], in1=bcast(sign_col),
                                op=Alu.mult)
        cur, nxt = nxt, cur

    def part_straight(d):
        part_sub(s_sign[d], shuffle_mask(lambda i: (i & 16) | ((i % 16) ^ d)),
                 rev_free=False)

    def part_flip(kb):
        def pm(i):
            q, p = i & 16, i % 16
            blk = (p // kb) * kb
            return q | (blk + (kb - 1 - (p % kb)))
        part_sub(s_flip[kb], shuffle_mask(pm), rev_free=True)

    k = 2
    while k <= N:
        if k <= CW:
            free_flip(k)
        else:
            part_flip(k // CW)
        j = k // 4
        while j >= 1:
            if j >= CW:
                part_straight(j // CW)
            else:
                free_straight(j)
            j //= 2
        k *= 2

    srt = cur  # final sorted tile (desc), pads at tail
    scratch = nxt  # the other ping-pong tile, now free

    # --- gumbel: tE(u) -> g in tE --------------------------------------
    # t1 = ln(u+1e-20) with series blend for u > 0.96
    g_act = scratch  # reuse: careful ordering -- scratch == nxt (free)
    nc.scalar.activation(g_act[:], tE[:], Act.Ln, bias=s_eps[:], scale=1.0)
    # series: d = u-1 ; t1s = d*(1 - d*(0.5 - d/3)) via Horner
    dtt = tC
    nc.vector.tensor_scalar(out=dtt[:], in0=tE[:], scalar1=-1.0, scalar2=None,
                            op0=Alu.add)                      # d = u - 1
    h = tE  # overwrite u progressively (blend mask computed later from d)
    # h1 = -d/3 + 0.5
    nc.vector.tensor_scalar(out=h[:], in0=dtt[:], scalar1=-(1.0 / 3.0),
                            scalar2=0.5, op0=Alu.mult, op1=Alu.add)
    # h2 = 1 - d*h1  = (d mult h1)*-1 + 1
    nc.vector.tensor_tensor(out=h[:], in0=dtt[:], in1=h[:], op=Alu.mult)
    nc.vector.tensor_scalar(out=h[:], in0=h[:], scalar1=-1.0, scalar2=1.0,
                            op0=Alu.mult, op1=Alu.add)
    # t1s = d*h2
    nc.vector.tensor_tensor(out=h[:], in0=dtt[:], in1=h[:], op=Alu.mult)
    # mask m = (d > -0.04) -> use series; blend arithmetically:
    # t1 = lnact + m*(t1s - lnact)
    nc.vector.tensor_scalar(out=dtt[:], in0=dtt[:], scalar1=-0.04, scalar2=None,
                            op0=Alu.is_gt)
    nc.vector.tensor_tensor(out=h[:], in0=h[:], in1=g_act[:], op=Alu.subtract)
    nc.vector.tensor_tensor(out=h[:], in0=h[:], in1=dtt[:], op=Alu.mult)
    nc.vector.tensor_tensor(out=h[:], in0=h[:], in1=g_act[:], op=Alu.add)  # t1 in tE
    # t2 = ln(-t1 + 1e-20); g = -t2  (fold into score later)
    nc.scalar.activation(tE[:], tE[:], Act.Ln, bias=s_eps[:], scale=-1.0)  # t2

    # --- exp/cumsum/threshold ------------------------------------------
    # m = row max = butterfly-max of srt[:,0:1]
    bfly(sc["m"], srt[:, 0:1], Alu.max, sc["t0"])
    nc.vector.tensor_scalar(out=sc["negm"][:], in0=sc["m"][:], scalar1=-1.0,
                            scalar2=None, op0=Alu.mult)
    e_t = tC
    nc.scalar.activation(e_t[:], srt[:], Act.Exp, bias=sc["negm"][:], scale=1.0)
    cum = scratch  # overwrite g_act (t1 consumed into tE)
    nc.vector.tensor_tensor_scan(cum[:], e_t[:], e_t[:], 0.0, Alu.add, Alu.bypass)
    # partition carry: tot = cum[:, -1:]
    nc.vector.tensor_copy(sc["tot"][:], cum[:, CW - 1:CW])
    # inclusive prefix across group partitions (masked shifts)
    nc.vector.tensor_copy(sc["pref"][:], sc["tot"][:])
    for s in (1, 2, 4, 8):
        nc.vector.stream_shuffle(sc["t0"][:], sc["pref"][:],
                                 shuffle_mask(lambda i: max(i - s, 0)))
        nc.vector.tensor_tensor(out=sc["t0"][:], in0=sc["t0"][:],
                                in1=s_cmask[s][:], op=Alu.mult)
        nc.vector.tensor_tensor(out=sc["pref"][:], in0=sc["pref"][:],
                                in1=sc["t0"][:], op=Alu.add)
    # carry = pref - tot ; Z = butterfly-sum(tot)
    nc.vector.tensor_tensor(out=sc["carry"][:], in0=sc["pref"][:],
                            in1=sc["tot"][:], op=Alu.subtract)
    bfly(sc["Z"], sc["tot"], Alu.add, sc["t0"])
    nc.vector.tensor_scalar(out=sc["thr"][:], in0=sc["Z"][:], scalar1=0.9,
                            scalar2=None, op0=Alu.mult)
    # cum += carry (broadcast)
    nc.vector.tensor_tensor(out=cum[:], in0=cum[:], in1=bcast(sc["carry"]),
                            op=Alu.add)

    # --- score = srt - t2, masked by kept ------------------------------
    nc.vector.tensor_tensor(out=tE[:], in0=srt[:], in1=tE[:], op=Alu.subtract)
    km = e_t  # reuse tC
    nc.vector.tensor_tensor(out=km[:], in0=cum[:], in1=bcast(sc["thr"]),
                            op=Alu.is_le)                     # 1.0 kept
    nc.vector.tensor_scalar(out=km[:], in0=km[:], scalar1=1.0e30, scalar2=-1.0e30,
                            op0=Alu.mult, op1=Alu.add)        # 0 or -1e30
    nc.vector.tensor_tensor(out=tE[:], in0=tE[:], in1=km[:], op=Alu.add)

    # --- idx tile (linear index within row, f32) -----------------------
    idx = tC  # overwrite km (consumed)
    nc.gpsimd.iota(idx[:].bitcast(i32), pattern=[[1, CW]], base=0,
                   channel_multiplier=CW)
    nc.vector.tensor_copy(idx[:], idx[:].bitcast(i32))  # i32 -> f32 in place
    nc.vector.tensor_tensor(out=idx[:], in0=idx[:], in1=bcast(s_rowbase),
                            op=Alu.subtract)

    # --- global argmax --------------------------------------------------
    nc.vector.tensor_reduce(sc["t1"][:], tE[:], axis=mybir.AxisListType.X,
                            op=Alu.max)
    bfly(sc["M"], sc["t1"], Alu.max, sc["t0"])
    # eq mask -> scratch(cum consumed)
    eq = cum
    nc.vector.tensor_tensor(out=eq[:], in0=tE[:], in1=bcast(sc["M"]),
                            op=Alu.is_equal)
    # r* = min over eq of idx : tmp = idx + (1-eq)*1e30
    tmp = tE  # score consumed
    nc.vector.tensor_scalar(out=tmp[:], in0=eq[:], scalar1=-1.0e30,
                            scalar2=1.0e30, op0=Alu.mult, op1=Alu.add)
    nc.vector.tensor_tensor(out=tmp[:], in0=tmp[:], in1=idx[:], op=Alu.add)
    nc.vector.tensor_reduce(sc["t1"][:], tmp[:], axis=mybir.AxisListType.X,
                            op=Alu.min)
    bfly(sc["rstar"], sc["t1"], Alu.min, sc["t0"])
    # x* = max over eq of srt : tmp = srt + (eq-1)*1e30
    nc.vector.tensor_scalar(out=tmp[:], in0=eq[:], scalar1=1.0e30,
                            scalar2=-1.0e30, op0=Alu.mult, op1=Alu.add)
    nc.vector.tensor_tensor(out=tmp[:], in0=tmp[:], in1=srt[:], op=Alu.add)
    nc.vector.tensor_reduce(sc["t1"][:], tmp[:], axis=mybir.AxisListType.X,
                            op=Alu.max)
    bfly(sc["xstar"], sc["t1"], Alu.max, sc["t0"])

    # --- token recovery in token space (tD) ----------------------------
    # cnt = #{x > x*}
    gt = tE
    nc.vector.tensor_tensor(out=gt[:], in0=tD[:], in1=bcast(sc["xstar"]),
                            op=Alu.is_gt)
    nc.vector.tensor_reduce(sc["t1"][:], gt[:], axis=mybir.AxisListType.X,
                            op=Alu.add)
    bfly(sc["cnt"], sc["t1"], Alu.add, sc["t0"])
    nc.vector.tensor_tensor(out=sc["jstar"][:], in0=sc["rstar"][:],
                            in1=sc["cnt"][:], op=Alu.subtract)
    # match mask
    mt = tE
    nc.vector.tensor_tensor(out=mt[:], in0=tD[:], in1=bcast(sc["xstar"]),
                            op=Alu.is_equal)
    # inclusive prefix count along free
    pc = eq  # scratch
    nc.vector.tensor_tensor_scan(pc[:], mt[:], mt[:], 0.0, Alu.add, Alu.bypass)
    # partition carry for pc
    nc.vector.tensor_copy(sc["tot"][:], pc[:, CW - 1:CW])
    nc.vector.tensor_copy(sc["pref"][:], sc["tot"][:])
    for s in (1, 2, 4, 8):
        nc.vector.stream_shuffle(sc["t0"][:], sc["pref"][:],
                                 shuffle_mask(lambda i: max(i - s, 0)))
        nc.vector.tensor_tensor(out=sc["t0"][:], in0=sc["t0"][:],
                                in1=s_cmask[s][:], op=Alu.mult)
        nc.vector.tensor_tensor(out=sc["pref"][:], in0=sc["pref"][:],
                                in1=sc["t0"][:], op=Alu.add)
    nc.vector.tensor_tensor(out=sc["carry"][:], in0=sc["pref"][:],
                            in1=sc["tot"][:], op=Alu.subtract)
    nc.vector.tensor_tensor(out=pc[:], in0=pc[:], in1=bcast(sc["carry"]),
                            op=Alu.add)
    # exclusive prefix = pc - mt ; w = mt * (pcex == j*)
    nc.vector.tensor_tensor(out=pc[:], in0=pc[:], in1=mt[:], op=Alu.subtract)
    nc.vector.tensor_tensor(out=pc[:], in0=pc[:], in1=bcast(sc["jstar"]),
                            op=Alu.is_equal)
    nc.vector.tensor_tensor(out=pc[:], in0=pc[:], in1=mt[:], op=Alu.mult)
    # token = min over w of idx
    nc.vector.tensor_scalar(out=pc[:], in0=pc[:], scalar1=-1.0e30,
                            scalar2=1.0e30, op0=Alu.mult, op1=Alu.add)
    nc.vector.tensor_tensor(out=pc[:], in0=pc[:], in1=idx[:], op=Alu.add)
    nc.vector.tensor_reduce(sc["t1"][:], pc[:], axis=mybir.AxisListType.X,
                            op=Alu.min)
    bfly(sc["tokv"], sc["t1"], Alu.min, sc["t0"])

    # --- outputs --------------------------------------------------------
    nc.sync.dma_start(out=tok_d, in_=sc["tokv"][:])
    dbg = sm.tile([128, 8], f32, tag="dbg")
    for j, kk in enumerate(("M", "rstar", "xstar", "cnt", "jstar", "Z", "m",
                            "carry")):
        nc.vector.tensor_copy(dbg[:, j:j + 1], sc[kk][:])
    nc.sync.dma_start(out=dbg_d, in_=dbg[:])


def host_consts():
    c = np.zeros((128, 16), dtype=np.float32)
    p16 = np.arange(128) % 16
    for j, d in enumerate((1, 2, 4, 8)):
        c[:, j] = np.where((p16 & d) == 0, 1.0, -1.0)
    for j, kb in enumerate((2, 4, 8, 16)):
        c[:, 4 + j] = np.where((p16 % kb) < kb // 2, 1.0, -1.0)
    for j, s in enumerate((1, 2, 4, 8)):
        c[:, 8 + j] = np.where(p16 >= s, 1.0, 0.0)
    c[:, 12] = (np.arange(128) // 16) * (GP * CW)  # rowbase: global iota offset
    return c


_built = {}


def build_nc():
    if "nc" in _built:
        return _built["nc"]
    from contextlib import ExitStack
    import concourse.bacc as bacc
    from concourse.tile import TileContext

    nc = bacc.Bacc("TRN2", target_bir_lowering=False, debug=False,
                   num_devices=NCORES)
    with TileContext(nc) as tc:
        with ExitStack() as ctx:
            _emit(nc, tc, ctx)
    nc.compile()
    _built["nc"] = nc
    return nc


def kernel(logits, u):
    logits = np.ascontiguousarray(np.asarray(logits), dtype=np.float32)
    u = np.ascontiguousarray(np.asarray(u), dtype=np.float32)
    assert logits.shape == (B, V) and u.shape == (B, V)
    from concourse.bass_utils import run_bass_kernel_spmd

    nc = build_nc()
    consts = host_consts()
    in_maps = [
        {"x": logits[c * NROW:(c + 1) * NROW], "u": u[c * NROW:(c + 1) * NROW],
         "consts": consts}
        for c in range(NCORES)
    ]
    res = run_bass_kernel_spmd(nc, in_maps, list(range(NCORES)))
    toks = np.concatenate(
        [res.results[c]["tok"][::GP, 0] for c in range(NCORES)])
    return np.round(toks).astype(np.int32)
